# revision 35
# baseline (speedup 1.0000x reference)
"""Causal attention with key padding for Trainium2, sharded over 8 NeuronCores.

Contract: kernel(**inputs) takes the FULL inputs (q, k, v, att_mask, pad_mask)
as numpy arrays and returns the FULL [B, H, L, D] output.

Strategy:
  - Shard the 64 (batch, head) units across 8 cores: core c gets units
    [8c, 8c+8), so each core sees a single batch's pad mask.
  - Host pre-transposes Q and K to [unit, D, L] (bf16); V is staged as
    [unit, L, D+1] with a ones column (softmax denominator) and padded key
    rows zeroed, which applies the key-padding mask for free.
  - Per head / query block, S^T[k, q] is computed chunk-by-chunk with bf16
    matmuls (fp32 PSUM), trimming fully-masked 128-col tiles of crossing
    chunks.  exp() is split across two engines: ScalarE activation (exact,
    bf16 out) and VectorE via a one-instruction Schraudolph exp
    (round(A*s + B) -> int16 bit pattern == bf16(exp(s/8))), balancing the
    two engine queues.  Causal boundary tiles get a [128,128] triangle
    multiply on GpSimd.
  - PV runs with P tiles as the stationary operand: out[q, d] accumulates
    P_tile^T @ [V | 1] per key chunk, so the denominator lands on the same
    partition as its query row.  A reciprocal + one broadcast multiply
    normalizes, and bf16 results DMA out (host converts to fp32).
"""

import numpy as np

N_CORES = 8
KC = 128          # key-chunk (partition) size
QB = 512          # query-block width
NCB = 3           # non-crossing chunks per exp batch (3 PSUM banks)
N_WARM = 120       # PE clock-gate warm-up matmuls

# Schraudolph exp constants: bits = round(A*s + B), s = raw score.
# bf16 variant (crossing chunks), fp8e4 variant (non-crossing chunks).
EXP_A = 128.0 / (8.0 * np.log(2.0))
EXP_B = 16256.0 - 128.0 * 0.02
# fp8 P is scaled down by K8 so exp(s/8) stays below e4m3's 240 max-finite
# (0x78+ encodes inf); vo8 carries 4*v and a 4.0 ones column to compensate.
K8 = 32.0
USE_FP8 = False    # DoubleRow PV is a net loss at FD=65; keep bf16 path
EXP_A8 = 8.0 / (8.0 * np.log(2.0))
EXP_B8 = 56.0 - 8.0 * 0.02 - 8.0 * np.log2(K8)


# --------------------------------------------------------------------------
# numpy fallback (exact reference math) -- only used if the input masks do
# not match the causal + suffix-pad structure this kernel specializes to.
# --------------------------------------------------------------------------
def _reference_np(q, k, v, att_mask, pad_mask):
    B, H, L, D = q.shape
    scale = np.float32(1.0) / np.sqrt(np.float32(D))
    out = np.empty_like(q)
    for b in range(B):
        for h in range(H):
            att = (q[b, h] @ k[b, h].T) * scale
            att = att + att_mask[0, 0]
            att = np.where(pad_mask[b][None, :], -np.inf, att)
            att = att - att.max(axis=-1, keepdims=True)
            p = np.exp(att)
            p = p / p.sum(axis=-1, keepdims=True)
            out[b, h] = p @ v[b, h]
    return out


# --------------------------------------------------------------------------
# batch/job schedule, shared by program builder (device) and host
# --------------------------------------------------------------------------
def _schedule(L, chunk_skip):
    """Per query block: list of exp batches.  Each batch is a dict with
    jobs [(kc, qs, width, slot)] and total width; crossing batches carry
    diag tile slots.  Slots are PSUM-bank aligned within [0, 1536)."""
    NCH = L // KC
    NQB = L // QB
    CPB = QB // KC
    per_qb = []
    for qb in range(NQB):
        batches = []
        ncs = [kc for kc in range(CPB * qb) if not chunk_skip[kc]]
        ngrp = (len(ncs) + NCB - 1) // NCB
        for i in range(0, len(ncs), NCB):
            grp = ncs[i : i + NCB]
            jobs = [(kc, 0, QB, j * QB) for j, kc in enumerate(grp)]
            # alternate exp engines across NC batches (VectorE first); with
            # even counts the last batch of a query block -- which gates the
            # next block's QK through the stg WAR -- lands on ScalarE.
            batches.append({"jobs": jobs, "width": len(grp) * QB,
                            "diag": [], "crossing": False,
                            "act": ngrp >= 2 and (i // NCB) % 2 == 1})
        # crossing chunks j=0..CPB-1 -> kc = CPB*qb + j, cols [j*KC, QB)
        cjobs, diag = [], []
        slot_for_j = {0: 0, 1: QB, 2: 2 * QB, 3: QB + 384}
        for j in range(CPB):
            kc = CPB * qb + j
            if kc >= NCH or chunk_skip[kc]:
                continue
            w = QB - j * KC
            s = slot_for_j[j]
            cjobs.append((kc, j * KC, w, s))
            diag.append(s)  # diag tile is the first KC cols of the job
        # crossing batch first: its exp and triangle masks are off the
        # critical path by the time PV (one batch delayed) needs them
        batches.insert(0, {"jobs": cjobs, "width": None,
                           "diag": diag, "crossing": True})
        per_qb.append(batches)
    return per_qb


# --------------------------------------------------------------------------
# Bass program builder
# --------------------------------------------------------------------------
def _build_program(NH, L, D, chunk_skip):
    import concourse.bacc as bacc
    import concourse.mybir as mybir
    import concourse.tile as tile

    f32 = mybir.dt.float32
    i16 = mybir.dt.int16
    u8 = mybir.dt.uint8
    f8 = mybir.dt.float8e4
    bf16 = mybir.dt.bfloat16
    ALU = mybir.AluOpType
    NCH = L // KC
    NQB = L // QB
    CPB = QB // KC
    DE = D + 1  # V plus ones column
    scale = float(1.0 / np.sqrt(np.float32(D)))
    per_qb = _schedule(L, chunk_skip)

    nc = bacc.Bacc("TRN2", target_bir_lowering=False, debug=False)

    qt_d = nc.dram_tensor("qt", [NH, D, L], bf16, kind="ExternalInput")
    kt_d = nc.dram_tensor("kt", [NH, D, L], bf16, kind="ExternalInput")
    vo_d = nc.dram_tensor("vo", [NH, KC, (L // KC) * DE], bf16,
                          kind="ExternalInput")
    vo8_d = (nc.dram_tensor("vo8", [NH, L, DE], f8, kind="ExternalInput")
             if USE_FP8 else None)
    tri_d = nc.dram_tensor("trimask", [KC, KC], bf16, kind="ExternalInput")
    # p-major layouts: one contiguous line per partition per DMA
    out_d = nc.dram_tensor("out", [NH, L // QB, KC, (QB // KC) * D], bf16,
                           kind="ExternalOutput")

    with tile.TileContext(nc) as tc:
        with (
            tc.tile_pool(name="consts", bufs=1) as consts,
            tc.tile_pool(name="ktp", bufs=6) as ktp,
            tc.tile_pool(name="qtp", bufs=6) as qtp,
            tc.tile_pool(name="vop", bufs=4) as vop,
            tc.tile_pool(name="vop8", bufs=4 if USE_FP8 else 1) as vop8,
            tc.tile_pool(name="ptp16", bufs=7) as ptp16,
            tc.tile_pool(name="ptp8", bufs=6 if USE_FP8 else 1) as ptp8,
            tc.tile_pool(name="osb", bufs=4) as osb,
            tc.tile_pool(name="recp", bufs=6) as recp,
            tc.tile_pool(name="stg", bufs=2, space="PSUM") as stgp,
            tc.tile_pool(name="acc", bufs=2, space="PSUM") as accp,
        ):
            tri = consts.tile([KC, KC], bf16)

            # Warm-up with no DMA dependency: sustained PE activity releases
            # the HAM clock gate, and a dummy exp pulls the ACT table load
            # off the critical path -- all while head 0 streams in.
            wsrc = consts.tile([KC, KC], bf16, tag="wsrc")
            nc.vector.memset(wsrc[:], 0.0)
            biask = consts.tile([KC, 1], f32, tag="biask")
            nc.vector.memset(biask[:], float(-np.log(K8)))
            bias0 = consts.tile([KC, 1], f32, tag="bias0")
            nc.vector.memset(bias0[:], 0.0)
            warm = accp.tile([KC, CPB, 2 * DE - 2], f32, tag="acc")
            wout = recp.tile([KC, CPB], f32, tag="rec")
            for i in range(N_WARM):
                nc.tensor.matmul(
                    out=warm[:, 0, 0:DE], lhsT=wsrc[:], rhs=wsrc[:, 0:DE],
                    start=True, stop=True,
                )
                if i == 0:
                    nc.scalar.activation(
                        out=wout[:, 0:1],
                        in_=warm[:, 0, 0:1],
                        func=mybir.ActivationFunctionType.Exp,
                    )

            nc.gpsimd.dma_start(out=tri[:], in_=tri_d[:])

            # Globally software-pipelined emission: PV for a query block is
            # emitted one batch after its last exp (PE never queues behind
            # exp), and each query block's epilogue one further batch later.
            # PV runs qt-major so each PSUM accumulation group is a
            # contiguous run of matmuls (interleaved open groups within one
            # bank are not safe).
            pv_queue = []   # [(h, qb, acc, pts, vo_t, o_sb)]
            epi_queue = []  # [(tick, h, qb, acc, o_sb)]
            tick = [0]

            def emit_pv_group(h, qb, acc, pts, vo_t, vo8_t, qt):
                # non-crossing chunks first within each accumulation group
                # (their exps complete earliest, crossing pt gets slack);
                # adjacent fp8 chunk pairs fuse into one DoubleRow matmul.
                pts = sorted(pts, key=lambda pb: pb[1]["crossing"])
                mms = []  # (pt, lhsT_builder args) closures are overkill
                for pt, batch in pts:
                    jobs = [j for j in batch["jobs"] if j[1] // KC <= qt]
                    if batch["crossing"]:
                        for kc, qs, w, slot in jobs:
                            off = slot + (qt - qs // KC) * KC
                            mms.append(("bf", pt, kc, off))
                    elif not USE_FP8:
                        for kc, qs, w, slot in jobs:
                            mms.append(("bf", pt, kc, slot + qt * KC))
                    else:
                        i = 0
                        while i < len(jobs):
                            kc, qs, w, slot = jobs[i]
                            if (i + 1 < len(jobs)
                                    and jobs[i + 1][0] == kc + 1
                                    and jobs[i + 1][3] == slot + QB):
                                mms.append(("dr", pt, kc, slot + qt * KC))
                                i += 2
                            else:
                                mms.append(("f8", pt, kc, slot + qt * KC))
                                i += 1
                for idx, (kind, pt, kc, off) in enumerate(mms):
                    start = idx == 0
                    stop = idx == len(mms) - 1
                    if kind == "dr":
                        lhsT = pt[:, off : off + 2 * QB].rearrange(
                            "p (j x) -> p j x", j=2
                        )[:, :, 0:KC]
                        nc.tensor.matmul(
                            out=acc[:, qt, 0:DE],
                            lhsT=lhsT,
                            rhs=vo8_t[:, kc : kc + 2, :],
                            start=start,
                            stop=stop,
                            perf_mode=mybir.MatmulPerfMode.DoubleRow,
                        )
                    else:
                        nc.tensor.matmul(
                            out=acc[:, qt, 0:DE],
                            lhsT=pt[:, off : off + KC],
                            rhs=(vo_t if kind == "bf" else vo8_t)[:, kc, :],
                            start=start,
                            stop=stop,
                        )

            def emit_epi(h, qb, acc, o_sb):
                rec = recp.tile([KC, CPB], f32)
                nc.vector.reciprocal(out=rec[:], in_=acc[:, :, D])
                nc.vector.tensor_tensor(
                    out=o_sb[:],
                    in0=acc[:, :, 0:D],
                    in1=rec[:, :, None].broadcast_to([KC, CPB, D]),
                    op=ALU.mult,
                )
                nc.gpsimd.dma_start(
                    out=out_d[h, qb].rearrange("p (j d) -> p j d", j=CPB),
                    in_=o_sb[:],
                )

            def flush(drain=False):
                # interleave PV with QK: pop up to two qt-groups per batch
                # slot so PV never forms one monolithic burst that delays
                # the current block's QK (and thus its exps).
                while epi_queue and (drain or epi_queue[0][0] < tick[0]):
                    _, h, qb, acc, o_sb = epi_queue.pop(0)
                    emit_epi(h, qb, acc, o_sb)
                npop = len(pv_queue) if drain else min(len(pv_queue), 2)
                for _ in range(npop):
                    t0, h, qb, acc, pts, vo_t, vo8_t, o_sb, qt = pv_queue[0]
                    if not drain and (t0 >= tick[0] - 1 or len(pv_queue) < 5):
                        break
                    pv_queue.pop(0)
                    emit_pv_group(h, qb, acc, pts, vo_t, vo8_t, qt)
                    if qt == CPB - 1:
                        epi_queue.append((tick[0], h, qb, acc, o_sb))

            for h in range(NH):
                # K^T and Q^T duplicated into both partition halves so QK^T
                # matmuls run row-packed (contract D=64 is half the array).
                kt_t = ktp.tile([2 * D, L], bf16)
                qt_t = qtp.tile([2 * D, L], bf16)
                vo_t = vop.tile([KC, NCH, DE], bf16)
                for lo, hi in ((0, QB), (QB, L)):
                    nc.sync.dma_start(out=kt_t[0:D, lo:hi], in_=kt_d[h, :, lo:hi])
                    nc.sync.dma_start(out=qt_t[0:D, lo:hi], in_=qt_d[h, :, lo:hi])
                    nc.sync.dma_start(
                        out=kt_t[D : 2 * D, lo:hi], in_=kt_d[h, :, lo:hi]
                    )
                    nc.sync.dma_start(
                        out=qt_t[D : 2 * D, lo:hi], in_=qt_d[h, :, lo:hi]
                    )
                nc.gpsimd.dma_start(
                    out=vo_t[:],
                    in_=vo_d[h].rearrange("p (c e) -> p c e", e=DE),
                )
                vo8_t = None
                if USE_FP8:
                    vo8_t = vop8.tile([KC, NCH, DE], f8)
                    nc.gpsimd.dma_start(
                        out=vo8_t[:],
                        in_=vo8_d[h].rearrange("(c p) e -> p c e", p=KC),
                    )

                qb_order = (
                    list(reversed(range(NQB))) if h == NH - 1 else range(NQB)
                )
                for qb in qb_order:
                    # padded to 128 floats per qt so the tile is exactly one
                    # PSUM bank and no matmul output crosses a bank boundary
                    acc = accp.tile([KC, CPB, 2 * DE - 2], f32, tag="acc")
                    o_sb = osb.tile([KC, CPB, D], bf16)
                    pts = []
                    for batch in per_qb[qb]:
                        jobs = batch["jobs"]
                        stg = stgp.tile([KC, NCB * QB], f32)
                        for i, (kc, qs, w, slot) in enumerate(jobs):
                            half = i % 2  # row-group for 2x packing
                            nc.tensor.matmul(
                                out=stg[:, slot : slot + w],
                                lhsT=kt_t[
                                    half * D : (half + 1) * D,
                                    kc * KC : (kc + 1) * KC,
                                ],
                                rhs=qt_t[
                                    half * D : (half + 1) * D,
                                    qb * QB + qs : (qb + 1) * QB,
                                ],
                                start=True,
                                stop=True,
                            )
                        if batch["crossing"] or not USE_FP8:
                            pt = ptp16.tile([KC, NCB * QB], bf16)
                        else:
                            pt = ptp8.tile([KC, NCB * QB], f8)
                        # merged contiguous column runs covered by the jobs
                        ivals = sorted((s, s + w) for _, _, w, s in jobs)
                        runs = [list(ivals[0])]
                        for s, e in ivals[1:]:
                            if s == runs[-1][1]:
                                runs[-1][1] = e
                            else:
                                runs.append([s, e])
                        if (not batch["crossing"] and batch["act"]
                                and runs[0][1] > QB):
                            # ScalarE batches gate the next block's QK via
                            # the stg WAR: split so exp issues right after
                            # the first QK job (VectorE batches don't gate;
                            # splitting them only adds per-call overhead)
                            runs = [[0, QB], [QB, runs[0][1]]] + runs[1:]
                        use_act = batch["crossing"] or batch["act"]
                        for lo, hi in runs:
                            if use_act:
                                nc.scalar.activation(
                                    out=pt[:, lo:hi],
                                    in_=stg[:, lo:hi],
                                    func=mybir.ActivationFunctionType.Exp,
                                    scale=scale,
                                    bias=(biask[:] if USE_FP8
                                          and not batch["crossing"]
                                          else bias0[:]),
                                )
                            elif USE_FP8:
                                nc.vector.tensor_scalar(
                                    out=pt[:, lo:hi].bitcast(u8),
                                    in0=stg[:, lo:hi],
                                    scalar1=float(EXP_A8),
                                    scalar2=float(EXP_B8),
                                    op0=ALU.mult,
                                    op1=ALU.add,
                                )
                            else:
                                nc.vector.tensor_scalar(
                                    out=pt[:, lo:hi].bitcast(i16),
                                    in0=stg[:, lo:hi],
                                    scalar1=float(EXP_A),
                                    scalar2=float(EXP_B),
                                    op0=ALU.mult,
                                    op1=ALU.add,
                                )
                        for s in batch["diag"]:
                            nc.gpsimd.tensor_tensor(
                                out=pt[:, s : s + KC],
                                in0=pt[:, s : s + KC],
                                in1=tri[:],
                                op=ALU.mult,
                            )
                        pts.append((pt, batch))
                        tick[0] += 1
                        flush()
                    for qt in range(CPB):
                        pv_queue.append(
                            (tick[0], h, qb, acc, pts, vo_t, vo8_t, o_sb, qt)
                        )
            flush(drain=True)
            flush(drain=True)
    nc.finalize()
    return nc


# --------------------------------------------------------------------------
# host-side wrapper
# --------------------------------------------------------------------------
_PROG_CACHE = {}


def _get_program(NH, L, D, chunk_skip):
    key = (NH, L, D, tuple(chunk_skip))
    if key not in _PROG_CACHE:
        _PROG_CACHE[key] = _build_program(NH, L, D, chunk_skip)
    return _PROG_CACHE[key]


def _causal_ok(att_mask, L):
    if att_mask.shape != (1, 1, L, L):
        return False
    m = att_mask[0, 0]
    iu = np.triu_indices(L, 1)
    if not np.all(m[iu] == np.float32(-1e9)):
        return False
    il = np.tril_indices(L)
    return bool(np.all(m[il] == 0.0))


def kernel(q, k, v, att_mask, pad_mask):
    import ml_dtypes

    from concourse.bass_utils import run_bass_kernel_spmd

    B, H, L, D = q.shape
    U = B * H
    if (
        U % N_CORES != 0
        or L % QB != 0
        or D != 64
        or not _causal_ok(att_mask, L)
    ):
        return _reference_np(q, k, v, att_mask, pad_mask)

    NH = U // N_CORES  # units per core
    NCH = L // KC

    pad = np.asarray(pad_mask, dtype=bool)          # [B, L]
    pad_u = np.repeat(pad, H, axis=0)               # [U, L]
    # chunk skip list must be consistent across cores (single SPMD program)
    chunk_skip = [bool(np.all(pad_u[:, kc * KC : (kc + 1) * KC]))
                  for kc in range(NCH)]
    if chunk_skip[0] or np.any(pad_u[:, 0]):
        return _reference_np(q, k, v, att_mask, pad_mask)

    bf = ml_dtypes.bfloat16
    qf = np.ascontiguousarray(
        q.reshape(U, L, D).transpose(0, 2, 1)
    ).astype(bf)
    kf = np.ascontiguousarray(
        k.reshape(U, L, D).transpose(0, 2, 1)
    ).astype(bf)
    # V with ones column, padded key rows zeroed (applies the pad mask)
    vo = np.ones((U, L, D + 1), dtype=np.float32)
    vo[:, :, :D] = v.reshape(U, L, D)
    vo[pad_u] = 0.0
    vo8 = ((vo * np.float32(K8)).astype(ml_dtypes.float8_e4m3)
           if USE_FP8 else None)
    # p-major: [U, KC, NCH*(D+1)] with lines contiguous per partition
    vo = np.ascontiguousarray(
        vo.reshape(U, L // KC, KC, D + 1).transpose(0, 2, 1, 3)
    ).reshape(U, KC, (L // KC) * (D + 1)).astype(bf)

    tri = (np.arange(KC)[None, :] >= np.arange(KC)[:, None]).astype(bf)

    in_maps = []
    for c in range(N_CORES):
        sl = slice(c * NH, (c + 1) * NH)
        m = {"qt": qf[sl], "kt": kf[sl], "vo": vo[sl], "trimask": tri}
        if USE_FP8:
            m["vo8"] = vo8[sl]
        in_maps.append(m)

    nc = _get_program(NH, L, D, chunk_skip)
    import os

    kwargs = {}
    if os.environ.get("BASS_KERNEL_PROFILE") == "1":
        kwargs = dict(trace=True, trace_cores=[0], stitch_traces=False)
    res = run_bass_kernel_spmd(nc, in_maps, list(range(N_CORES)), **kwargs)
    global LAST_RESULT
    LAST_RESULT = res
    out = np.concatenate([np.asarray(r["out"]) for r in res.results], axis=0)
    # undo p-major layout: [U, NQB, KC, CPB, D] -> [U, NQB, CPB, KC, D]
    out = out.reshape(U, L // QB, KC, QB // KC, D).transpose(0, 1, 3, 2, 4)
    return np.ascontiguousarray(out).reshape(B, H, L, D).astype(
        np.float32, copy=False
    )


LAST_RESULT = None


# revision 36
# speedup vs baseline: 1.1401x; 1.1401x over previous
"""Causal attention with key padding for Trainium2, sharded over 8 NeuronCores.

Contract: kernel(**inputs) takes the FULL inputs (q, k, v, att_mask, pad_mask)
as numpy arrays and returns the FULL [B, H, L, D] output.

Strategy:
  - Shard the 64 (batch, head) units across 8 cores: core c gets units
    [8c, 8c+8), so each core sees a single batch's pad mask.
  - Host pre-transposes Q and K to [unit, D, L] (bf16); V is staged as
    [unit, L, D+1] with a ones column (softmax denominator) and padded key
    rows zeroed, which applies the key-padding mask for free.
  - Per head / query block, S^T[k, q] is computed chunk-by-chunk with bf16
    matmuls (fp32 PSUM), trimming fully-masked 128-col tiles of crossing
    chunks.  exp() is split across two engines: ScalarE activation (exact,
    bf16 out) and VectorE via a one-instruction Schraudolph exp
    (round(A*s + B) -> int16 bit pattern == bf16(exp(s/8))), balancing the
    two engine queues.  Causal boundary tiles get a [128,128] triangle
    multiply on GpSimd.
  - PV runs with P tiles as the stationary operand: out[q, d] accumulates
    P_tile^T @ [V | 1] per key chunk, so the denominator lands on the same
    partition as its query row.  A reciprocal + one broadcast multiply
    normalizes, and bf16 results DMA out (host converts to fp32).
"""

import numpy as np

N_CORES = 8
KC = 128          # key-chunk (partition) size
QB = 512          # query-block width
NCB = 3           # non-crossing chunks per exp batch (3 PSUM banks)
N_WARM = 120       # PE clock-gate warm-up matmuls

# Schraudolph exp constants: bits = round(A*s + B), s = raw score.
# bf16 variant (crossing chunks), fp8e4 variant (non-crossing chunks).
EXP_A = 128.0 / (8.0 * np.log(2.0))
EXP_B = 16256.0 - 128.0 * 0.02
# fp8 P is scaled down by K8 so exp(s/8) stays below e4m3's 240 max-finite
# (0x78+ encodes inf); vo8 carries 4*v and a 4.0 ones column to compensate.
K8 = 32.0
USE_FP8 = False    # DoubleRow PV is a net loss at FD=65; keep bf16 path
EXP_A8 = 8.0 / (8.0 * np.log(2.0))
EXP_B8 = 56.0 - 8.0 * 0.02 - 8.0 * np.log2(K8)


# --------------------------------------------------------------------------
# numpy fallback (exact reference math) -- only used if the input masks do
# not match the causal + suffix-pad structure this kernel specializes to.
# --------------------------------------------------------------------------
def _reference_np(q, k, v, att_mask, pad_mask):
    B, H, L, D = q.shape
    scale = np.float32(1.0) / np.sqrt(np.float32(D))
    out = np.empty_like(q)
    for b in range(B):
        for h in range(H):
            att = (q[b, h] @ k[b, h].T) * scale
            att = att + att_mask[0, 0]
            att = np.where(pad_mask[b][None, :], -np.inf, att)
            att = att - att.max(axis=-1, keepdims=True)
            p = np.exp(att)
            p = p / p.sum(axis=-1, keepdims=True)
            out[b, h] = p @ v[b, h]
    return out


# --------------------------------------------------------------------------
# batch/job schedule, shared by program builder (device) and host
# --------------------------------------------------------------------------
def _schedule(L, chunk_skip):
    """Per query block: list of exp batches.  Each batch is a dict with
    jobs [(kc, qs, width, slot)] and total width; crossing batches carry
    diag tile slots.  Slots are PSUM-bank aligned within [0, 1536)."""
    NCH = L // KC
    NQB = L // QB
    CPB = QB // KC
    per_qb = []
    for qb in range(NQB):
        batches = []
        ncs = [kc for kc in range(CPB * qb) if not chunk_skip[kc]]
        ngrp = (len(ncs) + NCB - 1) // NCB
        for i in range(0, len(ncs), NCB):
            grp = ncs[i : i + NCB]
            jobs = [(kc, 0, QB, j * QB) for j, kc in enumerate(grp)]
            # alternate exp engines across NC batches (VectorE first); with
            # even counts the last batch of a query block -- which gates the
            # next block's QK through the stg WAR -- lands on ScalarE.
            batches.append({"jobs": jobs, "width": len(grp) * QB,
                            "diag": [], "crossing": False,
                            "act": ngrp >= 2 and (i // NCB) % 2 == 1})
        # crossing chunks j=0..CPB-1 -> kc = CPB*qb + j, cols [j*KC, QB)
        cjobs, diag = [], []
        slot_for_j = {0: 0, 1: QB, 2: 2 * QB, 3: QB + 384}
        for j in range(CPB):
            kc = CPB * qb + j
            if kc >= NCH or chunk_skip[kc]:
                continue
            w = QB - j * KC
            s = slot_for_j[j]
            cjobs.append((kc, j * KC, w, s))
            diag.append(s)  # diag tile is the first KC cols of the job
        # crossing batch first: its exp and triangle masks are off the
        # critical path by the time PV (one batch delayed) needs them
        batches.insert(0, {"jobs": cjobs, "width": None,
                           "diag": diag, "crossing": True})
        per_qb.append(batches)
    return per_qb


# --------------------------------------------------------------------------
# Bass program builder
# --------------------------------------------------------------------------
def _build_program(NH, L, D, chunk_skip):
    import concourse.bacc as bacc
    import concourse.mybir as mybir
    import concourse.tile as tile

    f32 = mybir.dt.float32
    i16 = mybir.dt.int16
    u8 = mybir.dt.uint8
    f8 = mybir.dt.float8e4
    bf16 = mybir.dt.bfloat16
    ALU = mybir.AluOpType
    NCH = L // KC
    NQB = L // QB
    CPB = QB // KC
    DE = D + 1  # V plus ones column
    scale = float(1.0 / np.sqrt(np.float32(D)))
    per_qb = _schedule(L, chunk_skip)

    nc = bacc.Bacc("TRN2", target_bir_lowering=False, debug=False)

    qt_d = nc.dram_tensor("qt", [NH, D, L], bf16, kind="ExternalInput")
    kt_d = nc.dram_tensor("kt", [NH, D, L], bf16, kind="ExternalInput")
    vo_d = nc.dram_tensor("vo", [NH, KC, (L // KC) * DE], bf16,
                          kind="ExternalInput")
    vo8_d = (nc.dram_tensor("vo8", [NH, L, DE], f8, kind="ExternalInput")
             if USE_FP8 else None)
    tri_d = nc.dram_tensor("trimask", [KC, KC], bf16, kind="ExternalInput")
    # p-major layouts: one contiguous line per partition per DMA
    out_d = nc.dram_tensor("out", [NH, L // QB, KC, (QB // KC) * D], bf16,
                           kind="ExternalOutput")

    with tile.TileContext(nc) as tc:
        with (
            tc.tile_pool(name="consts", bufs=1) as consts,
            tc.tile_pool(name="ktp", bufs=6) as ktp,
            tc.tile_pool(name="qtp", bufs=6) as qtp,
            tc.tile_pool(name="vop", bufs=4) as vop,
            tc.tile_pool(name="vop8", bufs=4 if USE_FP8 else 1) as vop8,
            tc.tile_pool(name="ptp16", bufs=7) as ptp16,
            tc.tile_pool(name="ptp8", bufs=6 if USE_FP8 else 1) as ptp8,
            tc.tile_pool(name="osb", bufs=4) as osb,
            tc.tile_pool(name="recp", bufs=6) as recp,
            tc.tile_pool(name="stg", bufs=2, space="PSUM") as stgp,
            tc.tile_pool(name="acc", bufs=2, space="PSUM") as accp,
        ):
            tri = consts.tile([KC, KC], bf16)

            # Warm-up with no DMA dependency: sustained PE activity releases
            # the HAM clock gate, and a dummy exp pulls the ACT table load
            # off the critical path -- all while head 0 streams in.
            wsrc = consts.tile([KC, KC], bf16, tag="wsrc")
            nc.vector.memset(wsrc[:], 0.0)
            biask = consts.tile([KC, 1], f32, tag="biask")
            nc.vector.memset(biask[:], float(-np.log(K8)))
            bias0 = consts.tile([KC, 1], f32, tag="bias0")
            nc.vector.memset(bias0[:], 0.0)
            warm = accp.tile([KC, CPB, 2 * DE - 2], f32, tag="acc")
            wout = recp.tile([KC, CPB], f32, tag="rec")
            for i in range(N_WARM):
                nc.tensor.matmul(
                    out=warm[:, 0, 0:DE], lhsT=wsrc[:], rhs=wsrc[:, 0:DE],
                    start=True, stop=True,
                )
                if i == 0:
                    nc.scalar.activation(
                        out=wout[:, 0:1],
                        in_=warm[:, 0, 0:1],
                        func=mybir.ActivationFunctionType.Exp,
                    )

            nc.gpsimd.dma_start(out=tri[:], in_=tri_d[:])

            # Globally software-pipelined emission: PV for a query block is
            # emitted one batch after its last exp (PE never queues behind
            # exp), and each query block's epilogue one further batch later.
            # PV runs qt-major so each PSUM accumulation group is a
            # contiguous run of matmuls (interleaved open groups within one
            # bank are not safe).
            pv_queue = []   # [(h, qb, acc, pts, vo_t, o_sb)]
            epi_queue = []  # [(tick, h, qb, acc, o_sb)]
            tick = [0]

            def emit_pv_group(h, qb, acc, pts, vo_t, vo8_t, qt):
                # non-crossing chunks first within each accumulation group
                # (their exps complete earliest, crossing pt gets slack);
                # adjacent fp8 chunk pairs fuse into one DoubleRow matmul.
                pts = sorted(pts, key=lambda pb: pb[1]["crossing"])
                mms = []  # (pt, lhsT_builder args) closures are overkill
                for pt, batch in pts:
                    jobs = [j for j in batch["jobs"] if j[1] // KC <= qt]
                    if batch["crossing"]:
                        for kc, qs, w, slot in jobs:
                            off = slot + (qt - qs // KC) * KC
                            mms.append(("bf", pt, kc, off))
                    elif not USE_FP8:
                        for kc, qs, w, slot in jobs:
                            mms.append(("bf", pt, kc, slot + qt * KC))
                    else:
                        i = 0
                        while i < len(jobs):
                            kc, qs, w, slot = jobs[i]
                            if (i + 1 < len(jobs)
                                    and jobs[i + 1][0] == kc + 1
                                    and jobs[i + 1][3] == slot + QB):
                                mms.append(("dr", pt, kc, slot + qt * KC))
                                i += 2
                            else:
                                mms.append(("f8", pt, kc, slot + qt * KC))
                                i += 1
                for idx, (kind, pt, kc, off) in enumerate(mms):
                    start = idx == 0
                    stop = idx == len(mms) - 1
                    if kind == "dr":
                        lhsT = pt[:, off : off + 2 * QB].rearrange(
                            "p (j x) -> p j x", j=2
                        )[:, :, 0:KC]
                        nc.tensor.matmul(
                            out=acc[:, qt, 0:DE],
                            lhsT=lhsT,
                            rhs=vo8_t[:, kc : kc + 2, :],
                            start=start,
                            stop=stop,
                            perf_mode=mybir.MatmulPerfMode.DoubleRow,
                        )
                    else:
                        nc.tensor.matmul(
                            out=acc[:, qt, 0:DE],
                            lhsT=pt[:, off : off + KC],
                            rhs=(vo_t if kind == "bf" else vo8_t)[:, kc, :],
                            start=start,
                            stop=stop,
                        )

            def emit_epi(h, qb, acc, o_sb):
                rec = recp.tile([KC, CPB], f32)
                nc.vector.reciprocal(out=rec[:], in_=acc[:, :, D])
                nc.vector.tensor_tensor(
                    out=o_sb[:],
                    in0=acc[:, :, 0:D],
                    in1=rec[:, :, None].broadcast_to([KC, CPB, D]),
                    op=ALU.mult,
                )
                nc.gpsimd.dma_start(
                    out=out_d[h, qb].rearrange("p (j d) -> p j d", j=CPB),
                    in_=o_sb[:],
                )

            def flush(drain=False):
                # interleave PV with QK: pop up to two qt-groups per batch
                # slot so PV never forms one monolithic burst that delays
                # the current block's QK (and thus its exps).
                while epi_queue and (drain or epi_queue[0][0] < tick[0]):
                    _, h, qb, acc, o_sb = epi_queue.pop(0)
                    emit_epi(h, qb, acc, o_sb)
                npop = len(pv_queue) if drain else min(len(pv_queue), 2)
                for _ in range(npop):
                    t0, h, qb, acc, pts, vo_t, vo8_t, o_sb, qt = pv_queue[0]
                    if not drain and t0 >= tick[0]:
                        break
                    pv_queue.pop(0)
                    emit_pv_group(h, qb, acc, pts, vo_t, vo8_t, qt)
                    if qt == CPB - 1:
                        epi_queue.append((tick[0], h, qb, acc, o_sb))

            for h in range(NH):
                # K^T and Q^T duplicated into both partition halves so QK^T
                # matmuls run row-packed (contract D=64 is half the array).
                kt_t = ktp.tile([2 * D, L], bf16)
                qt_t = qtp.tile([2 * D, L], bf16)
                vo_t = vop.tile([KC, NCH, DE], bf16)
                for lo, hi in ((0, QB), (QB, L)):
                    nc.sync.dma_start(out=kt_t[0:D, lo:hi], in_=kt_d[h, :, lo:hi])
                    nc.sync.dma_start(out=qt_t[0:D, lo:hi], in_=qt_d[h, :, lo:hi])
                    nc.sync.dma_start(
                        out=kt_t[D : 2 * D, lo:hi], in_=kt_d[h, :, lo:hi]
                    )
                    nc.sync.dma_start(
                        out=qt_t[D : 2 * D, lo:hi], in_=qt_d[h, :, lo:hi]
                    )
                nc.gpsimd.dma_start(
                    out=vo_t[:],
                    in_=vo_d[h].rearrange("p (c e) -> p c e", e=DE),
                )
                vo8_t = None
                if USE_FP8:
                    vo8_t = vop8.tile([KC, NCH, DE], f8)
                    nc.gpsimd.dma_start(
                        out=vo8_t[:],
                        in_=vo8_d[h].rearrange("(c p) e -> p c e", p=KC),
                    )

                qb_order = (
                    list(reversed(range(NQB))) if h == NH - 1 else range(NQB)
                )
                for qb in qb_order:
                    # padded to 128 floats per qt so the tile is exactly one
                    # PSUM bank and no matmul output crosses a bank boundary
                    acc = accp.tile([KC, CPB, 2 * DE - 2], f32, tag="acc")
                    o_sb = osb.tile([KC, CPB, D], bf16)
                    pts = []
                    for batch in per_qb[qb]:
                        jobs = batch["jobs"]
                        stg = stgp.tile([KC, NCB * QB], f32)
                        for i, (kc, qs, w, slot) in enumerate(jobs):
                            half = i % 2  # row-group for 2x packing
                            nc.tensor.matmul(
                                out=stg[:, slot : slot + w],
                                lhsT=kt_t[
                                    half * D : (half + 1) * D,
                                    kc * KC : (kc + 1) * KC,
                                ],
                                rhs=qt_t[
                                    half * D : (half + 1) * D,
                                    qb * QB + qs : (qb + 1) * QB,
                                ],
                                start=True,
                                stop=True,
                            )
                        if batch["crossing"] or not USE_FP8:
                            pt = ptp16.tile([KC, NCB * QB], bf16)
                        else:
                            pt = ptp8.tile([KC, NCB * QB], f8)
                        # merged contiguous column runs covered by the jobs
                        ivals = sorted((s, s + w) for _, _, w, s in jobs)
                        runs = [list(ivals[0])]
                        for s, e in ivals[1:]:
                            if s == runs[-1][1]:
                                runs[-1][1] = e
                            else:
                                runs.append([s, e])
                        if (not batch["crossing"] and batch["act"]
                                and runs[0][1] > QB):
                            # ScalarE batches gate the next block's QK via
                            # the stg WAR: split so exp issues right after
                            # the first QK job (VectorE batches don't gate;
                            # splitting them only adds per-call overhead)
                            runs = [[0, QB], [QB, runs[0][1]]] + runs[1:]
                        use_act = batch["crossing"] or batch["act"]
                        for lo, hi in runs:
                            if use_act:
                                nc.scalar.activation(
                                    out=pt[:, lo:hi],
                                    in_=stg[:, lo:hi],
                                    func=mybir.ActivationFunctionType.Exp,
                                    scale=scale,
                                    bias=(biask[:] if USE_FP8
                                          and not batch["crossing"]
                                          else bias0[:]),
                                )
                            elif USE_FP8:
                                nc.vector.tensor_scalar(
                                    out=pt[:, lo:hi].bitcast(u8),
                                    in0=stg[:, lo:hi],
                                    scalar1=float(EXP_A8),
                                    scalar2=float(EXP_B8),
                                    op0=ALU.mult,
                                    op1=ALU.add,
                                )
                            else:
                                nc.vector.tensor_scalar(
                                    out=pt[:, lo:hi].bitcast(i16),
                                    in0=stg[:, lo:hi],
                                    scalar1=float(EXP_A),
                                    scalar2=float(EXP_B),
                                    op0=ALU.mult,
                                    op1=ALU.add,
                                )
                        for s in batch["diag"]:
                            nc.gpsimd.tensor_tensor(
                                out=pt[:, s : s + KC],
                                in0=pt[:, s : s + KC],
                                in1=tri[:],
                                op=ALU.mult,
                            )
                        pts.append((pt, batch))
                        tick[0] += 1
                        flush()
                    for qt in range(CPB):
                        pv_queue.append(
                            (tick[0], h, qb, acc, pts, vo_t, vo8_t, o_sb, qt)
                        )
            flush(drain=True)
            flush(drain=True)
    nc.finalize()
    return nc


# --------------------------------------------------------------------------
# host-side wrapper
# --------------------------------------------------------------------------
_PROG_CACHE = {}


def _get_program(NH, L, D, chunk_skip):
    key = (NH, L, D, tuple(chunk_skip))
    if key not in _PROG_CACHE:
        _PROG_CACHE[key] = _build_program(NH, L, D, chunk_skip)
    return _PROG_CACHE[key]


def _causal_ok(att_mask, L):
    if att_mask.shape != (1, 1, L, L):
        return False
    m = att_mask[0, 0]
    iu = np.triu_indices(L, 1)
    if not np.all(m[iu] == np.float32(-1e9)):
        return False
    il = np.tril_indices(L)
    return bool(np.all(m[il] == 0.0))


def kernel(q, k, v, att_mask, pad_mask):
    import ml_dtypes

    from concourse.bass_utils import run_bass_kernel_spmd

    B, H, L, D = q.shape
    U = B * H
    if (
        U % N_CORES != 0
        or L % QB != 0
        or D != 64
        or not _causal_ok(att_mask, L)
    ):
        return _reference_np(q, k, v, att_mask, pad_mask)

    NH = U // N_CORES  # units per core
    NCH = L // KC

    pad = np.asarray(pad_mask, dtype=bool)          # [B, L]
    pad_u = np.repeat(pad, H, axis=0)               # [U, L]
    # chunk skip list must be consistent across cores (single SPMD program)
    chunk_skip = [bool(np.all(pad_u[:, kc * KC : (kc + 1) * KC]))
                  for kc in range(NCH)]
    if chunk_skip[0] or np.any(pad_u[:, 0]):
        return _reference_np(q, k, v, att_mask, pad_mask)

    bf = ml_dtypes.bfloat16
    qf = np.ascontiguousarray(
        q.reshape(U, L, D).transpose(0, 2, 1)
    ).astype(bf)
    kf = np.ascontiguousarray(
        k.reshape(U, L, D).transpose(0, 2, 1)
    ).astype(bf)
    # V with ones column, padded key rows zeroed (applies the pad mask)
    vo = np.ones((U, L, D + 1), dtype=np.float32)
    vo[:, :, :D] = v.reshape(U, L, D)
    vo[pad_u] = 0.0
    vo8 = ((vo * np.float32(K8)).astype(ml_dtypes.float8_e4m3)
           if USE_FP8 else None)
    # p-major: [U, KC, NCH*(D+1)] with lines contiguous per partition
    vo = np.ascontiguousarray(
        vo.reshape(U, L // KC, KC, D + 1).transpose(0, 2, 1, 3)
    ).reshape(U, KC, (L // KC) * (D + 1)).astype(bf)

    tri = (np.arange(KC)[None, :] >= np.arange(KC)[:, None]).astype(bf)

    in_maps = []
    for c in range(N_CORES):
        sl = slice(c * NH, (c + 1) * NH)
        m = {"qt": qf[sl], "kt": kf[sl], "vo": vo[sl], "trimask": tri}
        if USE_FP8:
            m["vo8"] = vo8[sl]
        in_maps.append(m)

    nc = _get_program(NH, L, D, chunk_skip)
    import os

    kwargs = {}
    if os.environ.get("BASS_KERNEL_PROFILE") == "1":
        kwargs = dict(trace=True, trace_cores=[0], stitch_traces=False)
    res = run_bass_kernel_spmd(nc, in_maps, list(range(N_CORES)), **kwargs)
    global LAST_RESULT
    LAST_RESULT = res
    out = np.concatenate([np.asarray(r["out"]) for r in res.results], axis=0)
    # undo p-major layout: [U, NQB, KC, CPB, D] -> [U, NQB, CPB, KC, D]
    out = out.reshape(U, L // QB, KC, QB // KC, D).transpose(0, 1, 3, 2, 4)
    return np.ascontiguousarray(out).reshape(B, H, L, D).astype(
        np.float32, copy=False
    )


LAST_RESULT = None
